# revision 1
# baseline (speedup 1.0000x reference)
"""Trainium2 Bass kernel for nn_AdaptiveModFusion (gnn_message_passing).

Self-contained: takes FULL inputs, shards over 8 NeuronCores internally.

Strategy:
- Data-parallel over B: core c owns graphs 2c, 2c+1 (nodes [c*2048, (c+1)*2048)).
- Phase A (per-core): attention/gate/LN/conf/mixer in transposed [C, n] layout
  (weights stationary on PE; softmax via L^T + PE ones-matmul row sums).
- Phase B: AllGather of bf16 xn [16384, 256].
- Phase C: RGCN message passing: dma_gather of xn[src] rows + scatter via
  one-hot (weighted) matmuls into (dst,rel) segment sums G^T in PSUM, then
  relation matmuls + root + LN2 + conf-weighted pooling + head.
- Host prep (indices only): bucket edges by dst shard, sort by segment,
  pad seg tiles to 128 (sizes maxed over cores so the SPMD graph is uniform),
  build per-edge one-hot weight blocks P (1/cnt folded in) as bf16.
"""
import os
import sys

sys.path.insert(0, "/opt/trn_rl_repo")
import numpy as np

from concourse import bacc, bass, mybir, tile
from concourse.bass_utils import run_bass_kernel_spmd

B, N, C, R = 16, 1024, 256, 8
NUM_SUPER = 32
EPS = 1e-5
NCORES = 8
GPC = B // NCORES          # graphs per core = 2
NPC = GPC * N              # nodes per core = 2048
SEGS = NPC * R             # segments per core = 16384
NSEGT = SEGS // 128        # seg tiles per core = 128
NDB = NPC // 128           # dst blocks per core = 16
STPB = NSEGT // NDB        # seg tiles per dst block = 8

F32 = mybir.dt.float32
BF16 = mybir.dt.bfloat16
I16 = mybir.dt.int16
BF = mybir.dt.np(BF16)
AF = mybir.ActivationFunctionType
OP = mybir.AluOpType


def _host_prep(inputs):
    """Shard + layout inputs per core. Index-only compute on host."""
    f = {k: np.asarray(v) for k, v in inputs.items()}
    x = f["x"].astype(np.float32)
    src = np.asarray(f["batch_edge_index"][0], np.int64)
    dst = np.asarray(f["batch_edge_index"][1], np.int64)
    et = np.asarray(f["batch_edge_types"], np.int64)

    core_of = dst // NPC
    seg_g = dst * R + et
    cntg = np.bincount(seg_g, minlength=B * N * R).astype(np.float32)
    w_all = 1.0 / np.maximum(cntg[seg_g], 1.0)

    per_core = []
    counts = np.zeros((NCORES, NSEGT), np.int64)
    for c in range(NCORES):
        sel = core_of == c
        s_c, d_c, w_c = src[sel], dst[sel] - c * NPC, w_all[sel]
        seg_c = d_c * R + et[sel]
        order = np.argsort(seg_c, kind="stable")
        s_c, seg_c, w_c = s_c[order], seg_c[order], w_c[order]
        t_c = seg_c // 128
        counts[c] = np.bincount(t_c, minlength=NSEGT)
        per_core.append((s_c, seg_c, w_c, t_c))

    szt = np.maximum(128, ((counts.max(0) + 127) // 128) * 128)  # [NSEGT]
    starts = np.zeros(NSEGT + 1, np.int64)
    starts[1:] = np.cumsum(szt)
    TOT = int(starts[-1])
    NBLK = TOT // 128
    db_off = [int(starts[b * STPB]) for b in range(NDB)]
    db_sz = [int(starts[(b + 1) * STPB] - starts[b * STPB]) for b in range(NDB)]
    szt_blocks = (szt // 128).astype(np.int64)

    meta = dict(TOT=TOT, NBLK=NBLK, db_off=db_off, db_sz=db_sz,
                tile_start=starts[:-1].astype(np.int64), szt_blocks=szt_blocks)

    idx_all, P_all = [], []
    for c in range(NCORES):
        s_c, seg_c, w_c, t_c = per_core[c]
        M = len(s_c)
        grp_first = np.searchsorted(t_c, np.arange(NSEGT), "left")
        within = np.arange(M) - grp_first[t_c]
        pos = starts[t_c] + within
        idx16 = np.zeros(TOT, np.int16)
        idx16[pos] = s_c.astype(np.int16)
        P = np.zeros((TOT, 128), BF)
        P[pos, seg_c % 128] = w_c.astype(BF)
        iw = np.ascontiguousarray(idx16.reshape(TOT // 16, 16).T)  # [16, TOT//16]
        iw = np.tile(iw, (8, 1))                                   # [128, TOT//16]
        idx_all.append(iw)
        P_all.append(np.ascontiguousarray(P.reshape(NBLK, 128, 128)))

    def chunkw(w):  # [K, M] -> [128, K//128, M]
        K, M = w.shape
        return np.ascontiguousarray(
            w.astype(np.float32).reshape(K // 128, 128, M).transpose(1, 0, 2)
        ).astype(BF)

    shared = dict(
        Wq=chunkw(f["Wq"]), Wk=chunkw(f["Wk"]), Wv=chunkw(f["Wv"]),
        Wg=chunkw(f["Wg"]), Wc1=chunkw(f["Wc1"]),
        Wc2=f["Wc2"].astype(np.float32).astype(BF).reshape(128, 1),
        Wm=chunkw(f["Wm"]), Wroot=chunkw(f["W_root"]), Wh=chunkw(f["Wh"]),
        Wrel=np.ascontiguousarray(
            f["W_rel"].astype(np.float32).reshape(R, 2, 128, 256).transpose(2, 0, 1, 3)
        ).astype(BF),  # [128, R, 2, 256]
        bq=f["bq"].astype(np.float32).reshape(2, 128).T.copy(),
        bk=f["bk"].astype(np.float32).reshape(2, 128).T.copy(),
        bg=f["bg"].astype(np.float32).reshape(2, 128).T.copy(),
        bm=f["bm"].astype(np.float32).reshape(2, 128).T.copy(),
        bc1=f["bc1"].astype(np.float32).reshape(128, 1).copy(),
        bc2=f["bc2"].astype(np.float32).reshape(1, 1).copy(),
        bh=f["bh"].astype(np.float32).reshape(32, 1).copy(),
        bv_row=f["bv"].astype(np.float32).reshape(1, 256).astype(BF),
        brg_row=f["b_rgcn"].astype(np.float32).reshape(1, 256).astype(BF),
        g_lna=f["ln_attn_g"].astype(np.float32).reshape(2, 128).T.copy(),
        b_lna=f["ln_attn_b"].astype(np.float32).reshape(2, 128).T.copy(),
        g_ln1=f["ln1_g"].astype(np.float32).reshape(2, 128).T.copy(),
        b_ln1=f["ln1_b"].astype(np.float32).reshape(2, 128).T.copy(),
        g2bc=np.ascontiguousarray(np.broadcast_to(
            f["ln2_g"].astype(np.float32), (128, 256))).astype(BF),
        b2bc=np.ascontiguousarray(np.broadcast_to(
            f["ln2_b"].astype(np.float32), (128, 256))).astype(BF),
        ident=np.eye(128, dtype=BF),
        ones_row=np.ones((1, 128), BF),
        invC_col=np.full((128, 1), 1.0 / C, BF),
        ones_col=np.ones((128, 1), BF),
        ind=np.ascontiguousarray(np.stack(
            [np.tile([1, 0], (128, 1)), np.tile([0, 1], (128, 1))], 1
        ).reshape(128, 4)).astype(BF),
        eps1=np.full((1, 1), EPS, np.float32),
        eps128=np.full((128, 1), EPS, np.float32),
    )

    in_maps = []
    for c in range(NCORES):
        xT = np.concatenate([x[GPC * c + g].T for g in range(GPC)], axis=1)
        m = dict(shared)
        m["xT"] = np.ascontiguousarray(xT).astype(BF)
        m["idx"] = idx_all[c]
        m["P"] = P_all[c]
        in_maps.append(m)
    return in_maps, meta


def _build(meta, debug=False):
    nc = bacc.Bacc("TRN2", target_bir_lowering=False, debug=False,
                   num_devices=NCORES)
    core_ids = list(range(NCORES))
    TOT, NBLK = meta["TOT"], meta["NBLK"]

    def din(name, shape, dt):
        return nc.dram_tensor(name, shape, dt, kind="ExternalInput").ap()

    ios = {}
    for nm, shape, dt in [
        ("xT", [256, NPC], BF16), ("idx", [128, TOT // 16], I16),
        ("P", [NBLK, 128, 128], BF16),
        ("Wq", [128, 2, 256], BF16), ("Wk", [128, 2, 256], BF16),
        ("Wv", [128, 2, 256], BF16), ("Wg", [128, 4, 256], BF16),
        ("Wc1", [128, 2, 128], BF16), ("Wc2", [128, 1], BF16),
        ("Wm", [128, 4, 256], BF16), ("Wroot", [128, 2, 256], BF16),
        ("Wh", [128, 2, 32], BF16), ("Wrel", [128, R, 2, 256], BF16),
        ("bq", [128, 2], F32), ("bk", [128, 2], F32), ("bg", [128, 2], F32),
        ("bm", [128, 2], F32), ("bc1", [128, 1], F32), ("bc2", [1, 1], F32),
        ("bh", [32, 1], F32), ("bv_row", [1, 256], BF16),
        ("brg_row", [1, 256], BF16),
        ("g_lna", [128, 2], F32), ("b_lna", [128, 2], F32),
        ("g_ln1", [128, 2], F32), ("b_ln1", [128, 2], F32),
        ("g2bc", [128, 256], BF16), ("b2bc", [128, 256], BF16),
        ("ident", [128, 128], BF16), ("ones_row", [1, 128], BF16),
        ("invC_col", [128, 1], BF16), ("ones_col", [128, 1], BF16),
        ("ind", [128, 4], BF16), ("eps1", [1, 1], F32), ("eps128", [128, 1], F32),
    ]:
        ios[nm] = din(nm, shape, dt)

    ios["out"] = nc.dram_tensor("out", [NUM_SUPER, GPC], F32, kind="ExternalOutput").ap()
    dbg = {}
    if debug:
        for nm, shape, dt in [
            ("dbg_xn", [NPC, 256], BF16), ("dbg_att", [128, 2, N], BF16),
            ("dbg_valid", [128, 2, N], BF16), ("dbg_conf", [1, N], F32),
            ("dbg_gt", [128, 2, 1024], BF16), ("dbg_ypre", [128, 256], F32),
            ("dbg_pool", [128, 4], F32), ("dbg_den", [1, 2], F32),
            ("dbg_yc", [128, 256], BF16), ("dbg_rstd", [128, 4], F32),
            ("dbg_ssum", [128, 4], F32), ("dbg_ssq", [128, 4], F32),
        ]:
            dbg[nm[4:]] = nc.dram_tensor(nm, shape, dt, kind="ExternalOutput").ap()

    with tile.TileContext(nc) as tc:
        _graph_body(nc, tc, ios, meta, dbg)
    nc.compile()
    return nc, core_ids


def _graph_body(nc, tc, ios, meta, dbg):
    from contextlib import ExitStack
    TOT, NBLK = meta["TOT"], meta["NBLK"]
    db_off, db_sz = meta["db_off"], meta["db_sz"]
    tile_start, szt_blocks = meta["tile_start"], meta["szt_blocks"]
    es = ExitStack()
    cst = es.enter_context(tc.tile_pool(name="cst", bufs=1))
    keep = es.enter_context(tc.tile_pool(name="keep", bufs=2))
    dram = es.enter_context(tc.tile_pool(name="dram", bufs=1, space="DRAM"))
    esA = ExitStack()
    pa = esA.enter_context(tc.tile_pool(name="pa", bufs=1))
    ps = esA.enter_context(tc.tile_pool(name="ps", bufs=1, space="PSUM"))

    MM = nc.tensor.matmul

    def T(pool, shape, dt, nm, bufs=None):
        return pool.tile(shape, dt, name=nm, tag=nm, bufs=bufs)

    def ld(name, shape, dt):
        t = T(cst, shape, dt, f"sb_{name}")
        nc.sync.dma_start(t[:], ios[name][:])
        return t

    Wq = ld("Wq", [128, 2, 256], BF16); Wk = ld("Wk", [128, 2, 256], BF16)
    Wv = ld("Wv", [128, 2, 256], BF16); Wg = ld("Wg", [128, 4, 256], BF16)
    Wc1 = ld("Wc1", [128, 2, 128], BF16); Wc2 = ld("Wc2", [128, 1], BF16)
    Wm = ld("Wm", [128, 4, 256], BF16); Wroot = ld("Wroot", [128, 2, 256], BF16)
    Wh = ld("Wh", [128, 2, 32], BF16); Wrel = ld("Wrel", [128, R, 2, 256], BF16)
    bq = ld("bq", [128, 2], F32); bk = ld("bk", [128, 2], F32)
    bg = ld("bg", [128, 2], F32); bm = ld("bm", [128, 2], F32)
    bc1 = ld("bc1", [128, 1], F32); bc2 = ld("bc2", [1, 1], F32)
    bh = ld("bh", [32, 1], F32)
    bv_row = ld("bv_row", [1, 256], BF16); brg_row = ld("brg_row", [1, 256], BF16)
    g_lna = ld("g_lna", [128, 2], F32); b_lna = ld("b_lna", [128, 2], F32)
    g_ln1 = ld("g_ln1", [128, 2], F32); b_ln1 = ld("b_ln1", [128, 2], F32)
    g2bc = ld("g2bc", [128, 256], BF16); b2bc = ld("b2bc", [128, 256], BF16)
    ident = ld("ident", [128, 128], BF16)
    ones_row = ld("ones_row", [1, 128], BF16)
    invC_col = ld("invC_col", [128, 1], BF16)
    ones_col = ld("ones_col", [128, 1], BF16)
    ind = ld("ind", [128, 4], BF16)
    eps1 = ld("eps1", [1, 1], F32); eps128 = ld("eps128", [128, 1], F32)
    idx_sb = ld("idx", [128, TOT // 16], I16)

    xn_loc = dram.tile([NPC, 256], BF16, name="xn_loc", tag="xn_loc")
    xn_full = dram.tile([NCORES * NPC, 256], BF16,
                        name="xn_full", tag="xn_full")

    xnT_l, confcol_l, confbf_l = [], [], []

    PA_STAGES = int(os.environ.get("PA_STAGES", "9"))
    if PA_STAGES == 0:
        z0 = T(pa, [128, 8, 256], BF16, "xn_nm")
        nc.gpsimd.memset(z0[:], 0.0)
        for g in range(GPC):
            nc.sync.dma_start(
                xn_loc[g * N:(g + 1) * N].rearrange("(nb p) c -> p nb c", p=128), z0[:])
        for _ in range(GPC):
            xnT = T(keep, [128, 2, N], BF16, "xnT")
            nc.gpsimd.memset(xnT[:], 0.0)
            xnT_l.append(xnT)
            conf_col = T(keep, [128, 8], F32, "conf_col")
            nc.gpsimd.memset(conf_col[:], 0.5)
            confcol_l.append(conf_col)
            conf_bf = T(keep, [128, 8], BF16, "conf_bf")
            nc.vector.tensor_copy(conf_bf[:], conf_col[:])
            confbf_l.append(conf_bf)
    # ---------------- Phase A ----------------
    for g in range(GPC) if PA_STAGES > 0 else []:
        xT = T(pa, [128, 2, N], BF16, "xT", bufs=2)
        for cc in range(2):
            nc.sync.dma_start(xT[:, cc, :],
                              ios["xT"][cc * 128:(cc + 1) * 128, g * N:(g + 1) * N])

        qT = T(pa, [128, 2, N], BF16, "qT", bufs=2)
        kT = T(pa, [128, 2, N], BF16, "kT", bufs=2)
        for dst_t, Wsb, bcol in ((qT, Wq, bq), (kT, Wk, bk)):
            for co in range(2):
                p = T(ps, [128, N], F32, "big", bufs=2)
                for ci in range(2):
                    for h in range(2):
                        MM(p[:, h * 512:(h + 1) * 512],
                           Wsb[:, ci, co * 128:(co + 1) * 128],
                           xT[:, ci, h * 512:(h + 1) * 512],
                           start=(ci == 0), stop=(ci == 1))
                nc.scalar.activation(dst_t[:, co, :], p[:], AF.Identity,
                                     bias=bcol[:, co:co + 1])

        v = T(pa, [128, 8, 256], BF16, "v", bufs=2)
        for mb in range(8):
            p = T(ps, [128, 256], F32, "big", bufs=2)
            for ci in range(2):
                MM(p[:], xT[:, ci, mb * 128:(mb + 1) * 128], Wv[:, ci, :],
                   start=(ci == 0), stop=False)
            MM(p[:], ones_row[:], bv_row[:], start=False, stop=True)
            nc.scalar.copy(v[:, mb, :], p[:])

        pT = T(pa, [128, 8, N], BF16, "pT")
        p_s = T(ps, [1, N], F32, "row", bufs=2)
        for mb in range(8):
            pL = T(ps, [128, N], F32, "big", bufs=2)
            for ci in range(2):
                for h in range(2):
                    MM(pL[:, h * 512:(h + 1) * 512],
                       kT[:, ci, mb * 128:(mb + 1) * 128],
                       qT[:, ci, h * 512:(h + 1) * 512],
                       start=(ci == 0), stop=(ci == 1))
            nc.scalar.activation(pT[:, mb, :], pL[:], AF.Exp, scale=0.0625)
            for h in range(2):
                MM(p_s[:, h * 512:(h + 1) * 512], ones_col[:],
                   pT[:, mb, h * 512:(h + 1) * 512],
                   start=(mb == 0), stop=(mb == 7), skip_group_check=True)
        recip = T(pa, [1, N], F32, "recip")
        nc.vector.reciprocal(recip[:], p_s[:])
        rb = T(pa, [128, N], F32, "rb")
        nc.gpsimd.partition_broadcast(rb[:], recip[0:1, :])

        outT = T(pa, [128, 2, N], BF16, "outT")
        for co in range(2):
            po = T(ps, [128, N], F32, "big", bufs=2)
            for mb in range(8):
                for h in range(2):
                    MM(po[:, h * 512:(h + 1) * 512],
                       v[:, mb, co * 128:(co + 1) * 128],
                       pT[:, mb, h * 512:(h + 1) * 512],
                       start=(mb == 0), stop=(mb == 7))
            nc.vector.tensor_mul(outT[:, co, :], po[:], rb[:])
        if dbg and g == 0:
            nc.sync.dma_start(dbg["att"][:], outT[:])

        gateT = T(pa, [128, 2, N], BF16, "gateT", bufs=2)
        rhs4 = [outT[:, 0], outT[:, 1], xT[:, 0], xT[:, 1]]
        for co in range(2):
            p = T(ps, [128, N], F32, "big", bufs=2)
            for ci in range(4):
                for h in range(2):
                    MM(p[:, h * 512:(h + 1) * 512],
                       Wg[:, ci, co * 128:(co + 1) * 128],
                       rhs4[ci][:, h * 512:(h + 1) * 512],
                       start=(ci == 0), stop=(ci == 3))
            nc.scalar.activation(gateT[:, co, :], p[:], AF.Sigmoid,
                                 bias=bg[:, co:co + 1])

        fusedT = T(pa, [128, 2, N], BF16, "fusedT", bufs=2)
        for cc in range(2):
            t1 = T(pa, [128, N], BF16, "t1", bufs=2)
            t2 = T(pa, [128, N], BF16, "t2", bufs=2)
            nc.vector.tensor_sub(t1[:], outT[:, cc, :], xT[:, cc, :])
            nc.vector.tensor_mul(t2[:], gateT[:, cc, :], t1[:])
            nc.vector.tensor_add(fusedT[:, cc, :], xT[:, cc, :], t2[:])

        def ln_t(srcT, g_sb, b_sb, nm):
            p_mu = T(ps, [1, N], F32, "row", bufs=2)
            p_m2 = T(ps, [1, N], F32, "row", bufs=2)
            for cc in range(2):
                sq = T(pa, [128, N], BF16, "sq", bufs=2)
                nc.scalar.square(sq[:], srcT[:, cc, :])
                for h in range(2):
                    MM(p_mu[:, h * 512:(h + 1) * 512], invC_col[:],
                       srcT[:, cc, h * 512:(h + 1) * 512],
                       start=(cc == 0), stop=(cc == 1), skip_group_check=True)
                    MM(p_m2[:, h * 512:(h + 1) * 512], invC_col[:],
                       sq[:, h * 512:(h + 1) * 512],
                       start=(cc == 0), stop=(cc == 1), skip_group_check=True)
            mu = T(pa, [1, N], F32, "mu")
            nc.scalar.copy(mu[:], p_mu[:])
            mu2 = T(pa, [1, N], F32, "mu2")
            nc.vector.tensor_mul(mu2[:], mu[:], mu[:])
            var = T(pa, [1, N], F32, "var")
            nc.vector.tensor_sub(var[:], p_m2[:], mu2[:])
            sd = T(pa, [1, N], F32, "sd")
            nc.scalar.activation(sd[:], var[:], AF.Sqrt, bias=eps1[:])
            A_ = T(pa, [1, N], F32, "A_")
            nc.vector.reciprocal(A_[:], sd[:])
            negB = T(pa, [1, N], F32, "negB")
            nc.vector.scalar_tensor_tensor(negB[:], mu[:], -1.0, A_[:],
                                           OP.mult, OP.mult)
            Abf = T(pa, [1, N], BF16, "Abf")
            nc.vector.tensor_copy(Abf[:], A_[:])
            Bbf = T(pa, [1, N], BF16, "Bbf")
            nc.vector.tensor_copy(Bbf[:], negB[:])
            Abc = T(pa, [128, N], BF16, "Abc")
            nc.gpsimd.partition_broadcast(Abc[:], Abf[0:1, :])
            Bbc = T(pa, [128, N], BF16, "Bbc")
            nc.gpsimd.partition_broadcast(Bbc[:], Bbf[0:1, :])
            dstT = T(pa, [128, 2, N], BF16, f"ln_{nm}")
            for cc in range(2):
                u = T(pa, [128, N], BF16, "ln_u", bufs=2)
                w_ = T(pa, [128, N], BF16, "ln_w", bufs=2)
                nc.vector.tensor_mul(u[:], srcT[:, cc, :], Abc[:])
                nc.vector.tensor_add(w_[:], u[:], Bbc[:])
                nc.scalar.activation(dstT[:, cc, :], w_[:], AF.Identity,
                                     bias=b_sb[:, cc:cc + 1], scale=g_sb[:, cc:cc + 1])
            return dstT

        if PA_STAGES < 3:
            xnT = T(keep, [128, 2, N], BF16, "xnT")
            for cc in range(2):
                nc.vector.tensor_copy(xnT[:, cc, :], fusedT[:, cc, :])
            xnT_l.append(xnT)
            conf_col = T(keep, [128, 8], F32, "conf_col")
            nc.gpsimd.memset(conf_col[:], 0.5)
            confcol_l.append(conf_col)
            conf_bf = T(keep, [128, 8], BF16, "conf_bf")
            nc.vector.tensor_copy(conf_bf[:], conf_col[:])
            confbf_l.append(conf_bf)
            xn_nm0 = T(pa, [128, 8, 256], BF16, "xn_nm")
            for nb in range(8):
                nc.vector.tensor_copy(xn_nm0[:, nb, :],
                                      xnT.rearrange("p a n -> p (a n)")[:, nb * 256:(nb + 1) * 256])
            nc.sync.dma_start(
                xn_loc[g * N:(g + 1) * N].rearrange("(nb p) c -> p nb c", p=128),
                xn_nm0[:])
            continue
        validT = ln_t(fusedT, g_lna, b_lna, "attn")
        if dbg and g == 0:
            nc.sync.dma_start(dbg["valid"][:], validT[:])

        # conf net
        ph = T(ps, [128, N], F32, "big", bufs=2)
        for ci in range(2):
            for h in range(2):
                MM(ph[:, h * 512:(h + 1) * 512], Wc1[:, ci, :],
                   validT[:, ci, h * 512:(h + 1) * 512],
                   start=(ci == 0), stop=(ci == 1))
        hT = T(pa, [128, N], BF16, "hT")
        nc.scalar.activation(hT[:], ph[:], AF.Relu, bias=bc1[:])
        p_c = T(ps, [1, N], F32, "row", bufs=2)
        for h in range(2):
            MM(p_c[:, h * 512:(h + 1) * 512], Wc2[:], hT[:, h * 512:(h + 1) * 512],
               start=True, stop=True)
        conf_f = T(pa, [1, N], F32, "conf_f")
        nc.scalar.activation(conf_f[:], p_c[:], AF.Sigmoid, bias=bc2[:])
        if dbg and g == 0:
            nc.sync.dma_start(dbg["conf"][:], conf_f[:])
        cfb = T(pa, [1, N], BF16, "cfb")
        nc.vector.tensor_copy(cfb[:], conf_f[:])
        conf_bc = T(pa, [128, N], BF16, "conf_bc")
        nc.gpsimd.partition_broadcast(conf_bc[:], cfb[0:1, :])
        confpad = T(pa, [128, N], BF16, "confpad")
        nc.gpsimd.memset(confpad[:], 0.0)
        nc.vector.tensor_copy(confpad[0:1, :], conf_f[:])
        conf_col = T(keep, [128, 8], F32, "conf_col")
        conf_bf = T(keep, [128, 8], BF16, "conf_bf")
        for nb in range(8):
            ptc = T(ps, [128, 128], BF16, "big", bufs=2)
            nc.tensor.transpose(ptc[:], confpad[:, nb * 128:(nb + 1) * 128], ident[:])
            nc.vector.tensor_copy(conf_col[:, nb:nb + 1], ptc[:, 0:1])
            nc.vector.tensor_copy(conf_bf[:, nb:nb + 1], ptc[:, 0:1])
        confcol_l.append(conf_col)
        confbf_l.append(conf_bf)
        weightedT = T(pa, [128, 2, N], BF16, "weightedT", bufs=2)
        for cc in range(2):
            nc.vector.tensor_mul(weightedT[:, cc, :], validT[:, cc, :], conf_bc[:])

        # mixer
        xmT = T(pa, [128, 2, N], BF16, "xmT", bufs=2)
        rhs4m = [xT[:, 0], xT[:, 1], weightedT[:, 0], weightedT[:, 1]]
        for co in range(2):
            p = T(ps, [128, N], F32, "big", bufs=2)
            for ci in range(4):
                for h in range(2):
                    MM(p[:, h * 512:(h + 1) * 512],
                       Wm[:, ci, co * 128:(co + 1) * 128],
                       rhs4m[ci][:, h * 512:(h + 1) * 512],
                       start=(ci == 0), stop=(ci == 3))
            nc.scalar.activation(xmT[:, co, :], p[:], AF.Relu, bias=bm[:, co:co + 1])

        lnoutT = ln_t(xmT, g_ln1, b_ln1, "ln1")
        xnT = T(keep, [128, 2, N], BF16, "xnT")
        for cc in range(2):
            nc.vector.tensor_copy(xnT[:, cc, :], lnoutT[:, cc, :])
        xnT_l.append(xnT)

        # node-major xn -> DRAM
        xn_nm = T(pa, [128, 8, 256], BF16, "xn_nm")
        for nb in range(8):
            pt = T(ps, [128, 2, 128], BF16, "big", bufs=2)
            for cc in range(2):
                nc.tensor.transpose(pt[:, cc, :], xnT[:, cc, nb * 128:(nb + 1) * 128],
                                    ident[:])
            nc.vector.tensor_copy(xn_nm[:, nb, :], pt.rearrange("p a b -> p (a b)"))
        nc.sync.dma_start(
            xn_loc[g * N:(g + 1) * N].rearrange("(nb p) c -> p nb c", p=128),
            xn_nm[:])

    esA.close()
    if dbg:
        nc.sync.dma_start(dbg["xn"][:], xn_loc[:])

    # ---------------- Phase B: AllGather ----------------
    if os.environ.get("SKIP_COLLECTIVE"):
        nc.sync.dma_start(xn_full[0:NPC], xn_loc[:])
    else:
        nc.gpsimd.collective_compute(
            "AllGather", OP.bypass,
            replica_groups=[list(range(NCORES))],
            ins=[xn_loc.opt()], outs=[xn_full.opt()])

    # ---------------- Phase C: RGCN ----------------
    if os.environ.get("PHASEA_ONLY"):
        pcl0 = es.enter_context(tc.tile_pool(name="pcl0", bufs=1))
        hz = pcl0.tile([NUM_SUPER, GPC], F32, name="hz", tag="hz")
        nc.gpsimd.memset(hz[:], 0.0)
        nc.sync.dma_start(ios["out"][:], hz[:])
        es.close()
        return
    pcl = es.enter_context(tc.tile_pool(name="pcl", bufs=2))
    ps2 = es.enter_context(tc.tile_pool(name="ps2", bufs=1, space="PSUM"))
    p_pool0 = T(ps2, [128, 2], F32, "ppool0")
    p_pool1 = T(ps2, [128, 2], F32, "ppool1")
    p_pools = [p_pool0, p_pool1]
    p_den = T(ps2, [1, 2], F32, "pden")
    psagg = stat_sum = stat_sq = None
    for b in range(NDB):
        g, nbl = b // STPB, b % STPB
        nbb = db_sz[b] // 128
        blk0 = db_off[b] // 128
        Mt = T(pcl, [128, nbb, 256], BF16, "Mt", bufs=3)
        if os.environ.get("SKIP_GATHER"):
            nc.sync.dma_start(Mt[:], xn_full[0:nbb * 128].rearrange("(a p) c -> p a c", p=128))
        else:
            CH_G = 1024
            for off in range(0, db_sz[b], CH_G):
                n_i = min(CH_G, db_sz[b] - off)
                o16 = (db_off[b] + off) // 16
                nc.gpsimd.dma_gather(
                    Mt[:, off // 128:(off + n_i) // 128, :], xn_full[:],
                    idx_sb[:, o16:o16 + n_i // 16], n_i, n_i, 256)
        Pt = T(pcl, [128, nbb, 128], BF16, "Pt", bufs=3)
        nc.sync.dma_start(Pt[:], ios["P"][blk0:blk0 + nbb].rearrange("b p s -> p b s"))

        gt = T(pcl, [128, 2, 1024], BF16, "gt", bufs=2)
        for half in range(2):
            for cc in range(2):
                pg = T(ps2, [128, 512], F32, "psgt", bufs=2)
                for st4 in range(4):
                    st = b * STPB + half * 4 + st4
                    kblk = int(szt_blocks[st])
                    bs = (int(tile_start[st]) - db_off[b]) // 128
                    for k in range(kblk):
                        MM(pg[:, st4 * 128:(st4 + 1) * 128],
                           Mt[:, bs + k, cc * 128:(cc + 1) * 128], Pt[:, bs + k, :],
                           start=(st4 == 0 and k == 0),
                           stop=(st4 == 3 and k == kblk - 1),
                           skip_group_check=True)
                if cc == 0:
                    nc.vector.tensor_copy(gt[:, cc, half * 512:(half + 1) * 512], pg[:])
                else:
                    nc.scalar.copy(gt[:, cc, half * 512:(half + 1) * 512], pg[:])
        if dbg and b == 0:
            nc.sync.dma_start(dbg["gt"][:], gt[:])

        j = b % 4
        if j == 0:
            psagg = T(ps2, [128, 4, 256], F32, "psagg")
            stat_sum = T(pcl, [128, 4], F32, "ssum", bufs=2)
            stat_sq = T(pcl, [128, 4], F32, "ssq", bufs=2)
        for r in range(R):
            for cc in range(2):
                MM(psagg[:, j, :], gt[:, cc, r::8], Wrel[:, r, cc, :],
                   start=(r == 0 and cc == 0), stop=False)
        for cc in range(2):
            MM(psagg[:, j, :], xnT_l[g][:, cc, nbl * 128:(nbl + 1) * 128],
               Wroot[:, cc, :], start=False, stop=False)
        MM(psagg[:, j, :], ones_row[:], brg_row[:], start=False, stop=True)
        if dbg and b == 0:
            yp_f = T(pcl, [128, 256], F32, "yp_f")
            nc.vector.tensor_copy(yp_f[:], psagg[:, 0, :])
            nc.sync.dma_start(dbg["ypre"][:], yp_f[:])
        scr = T(pcl, [128, 256], BF16, "scr", bufs=2)
        nc.scalar.activation(scr[:], psagg[:, j, :], AF.Copy,
                             accum_out=stat_sum[:, j:j + 1])
        scr2 = T(pcl, [128, 256], BF16, "scr2", bufs=2)
        nc.scalar.activation(scr2[:], psagg[:, j, :], AF.Square,
                             accum_out=stat_sq[:, j:j + 1])

        if j == 3:
            negmu = T(pcl, [128, 4], F32, "negmu", bufs=2)
            nc.vector.tensor_scalar_mul(negmu[:], stat_sum[:], -1.0 / C)
            mu2 = T(pcl, [128, 4], F32, "mu2c", bufs=2)
            nc.vector.tensor_mul(mu2[:], negmu[:], negmu[:])
            var = T(pcl, [128, 4], F32, "varc", bufs=2)
            nc.vector.scalar_tensor_tensor(var[:], stat_sq[:], 1.0 / C, mu2[:],
                                           OP.mult, OP.subtract)
            sd = T(pcl, [128, 4], F32, "sdc", bufs=2)
            nc.scalar.activation(sd[:], var[:], AF.Sqrt, bias=eps128[:])
            rstd = T(pcl, [128, 4], F32, "rstdc", bufs=2)
            nc.vector.reciprocal(rstd[:], sd[:])
            negB = T(pcl, [128, 4], F32, "negBc", bufs=2)
            nc.vector.tensor_mul(negB[:], negmu[:], rstd[:])
            if dbg and b == 3:
                nc.sync.dma_start(dbg["rstd"][:], rstd[:])
                nc.sync.dma_start(dbg["ssum"][:], stat_sum[:])
                nc.sync.dma_start(dbg["ssq"][:], stat_sq[:])
            for jj in range(4):
                bb = b - 3 + jj
                gg, nb2 = bb // STPB, bb % STPB
                t2c = T(pcl, [128, 256], BF16, "t2c", bufs=4)
                nc.scalar.activation(t2c[:], psagg[:, jj, :], AF.Identity,
                                     bias=negB[:, jj:jj + 1], scale=rstd[:, jj:jj + 1])
                t3c = T(pcl, [128, 256], BF16, "t3c", bufs=4)
                nc.vector.tensor_mul(t3c[:], t2c[:], g2bc[:])
                t4c = T(pcl, [128, 256], BF16, "t4c", bufs=4)
                nc.vector.tensor_add(t4c[:], t3c[:], b2bc[:])
                yc = T(pcl, [128, 256], BF16, "ycc", bufs=4)
                nc.scalar.activation(yc[:], t4c[:], AF.Relu,
                                     scale=confcol_l[gg][:, nb2:nb2 + 1])
                if dbg and bb == 0:
                    nc.sync.dma_start(dbg["yc"][:], yc[:])
                for cc in range(2):
                    MM(p_pools[cc][:], yc[:, cc * 128:(cc + 1) * 128],
                       ind[:, gg * 2:gg * 2 + 2], start=(bb == 0), stop=(bb == NDB - 1),
                       skip_group_check=True)
                MM(p_den[:], confbf_l[gg][:, nb2:nb2 + 1], ind[:, gg * 2:gg * 2 + 2],
                   start=(bb == 0), stop=(bb == NDB - 1), skip_group_check=True)

    # ---------------- epilogue ----------------
    if dbg:
        pool_f = T(pcl, [128, 4], F32, "pool_f")
        nc.vector.tensor_copy(pool_f[:, 0:2], p_pool0[:])
        nc.vector.tensor_copy(pool_f[:, 2:4], p_pool1[:])
        nc.sync.dma_start(dbg["pool"][:], pool_f[:])
        den_f = T(pcl, [1, 2], F32, "den_f")
        nc.vector.tensor_copy(den_f[:], p_den[:])
        nc.sync.dma_start(dbg["den"][:], den_f[:])
    denm = T(pcl, [1, 2], F32, "denm")
    nc.vector.tensor_scalar_max(denm[:], p_den[:], 1e-8)
    denr = T(pcl, [1, 4], F32, "denr")
    nc.vector.reciprocal(denr[:, 0:2], denm[:])
    nc.vector.tensor_copy(denr[:, 2:4], denr[:, 0:2])
    drb = T(pcl, [128, 4], F32, "drb")
    nc.gpsimd.partition_broadcast(drb[:], denr[0:1, :])
    pn = T(pcl, [128, 4], BF16, "pn")
    nc.vector.tensor_mul(pn[:, 0:2], p_pool0[:], drb[:, 0:2])
    nc.vector.tensor_mul(pn[:, 2:4], p_pool1[:], drb[:, 2:4])
    p_head = T(ps2, [NUM_SUPER, GPC], F32, "phead")
    for cc in range(2):
        MM(p_head[:], Wh[:, cc, :], pn[:, cc * 2:cc * 2 + 2],
           start=(cc == 0), stop=(cc == 1))
    head_sb = T(pcl, [NUM_SUPER, GPC], F32, "head_sb")
    nc.scalar.activation(head_sb[:], p_head[:], AF.Identity, bias=bh[:])
    nc.sync.dma_start(ios["out"][:], head_sb[:])
    es.close()


_CACHE = {}


def kernel(**inputs):
    in_maps, meta = _host_prep(inputs)
    debug = bool(int(os.environ.get("KERNEL_DEBUG", "0")))
    key = (meta["TOT"], debug, os.environ.get("SKIP_COLLECTIVE"),
           os.environ.get("SKIP_GATHER"), os.environ.get("PHASEA_ONLY"),
           os.environ.get("PA_STAGES"))
    if key not in _CACHE:
        _CACHE[key] = _build(meta, debug=debug)
    nc, core_ids = _CACHE[key]
    res = run_bass_kernel_spmd(nc, in_maps, core_ids)
    out = np.zeros((B, NUM_SUPER), np.float32)
    for c in range(NCORES):
        oc = res.results[c]["out"]  # [32, 2]
        for g in range(GPC):
            out[GPC * c + g] = oc[:, g]
    if debug:
        kernel._dbg = res.results
    return out

